# revision 15
# baseline (speedup 1.0000x reference)
"""Trainium2 Bass kernel for AttnApply (sliding-window weighted sum).

out[b, t, c] = sum_i padded[b, t+i, c] * weights[b, t, i]   (T=11, D=5 zero pad)

Strategy
--------
Pure data parallel over batch: 8 cores x 4 batches each.

Per core, the windowed sum is a banded matrix multiply on the TensorEngine.
For a time block of M=118 output rows starting at t0 (K = M+T-1 = 128):

    out[t0+m, c] = sum_k band[k, m] * in_pad[t0+k, c],   k in [0, 128)

with band[k, m] = w[t0+m, k-m] for 0 <= k-m < T (zero elsewhere); in_pad is
host zero-padded so edge blocks need no special casing.  Band matrices are
built host-side (cheap scatter of the small weights tensor).

The matmul runs with the INPUT tile as the stationary operand and the band as
the moving operand, producing the TRANSPOSED output in PSUM:

    psum[c, m] = sum_k in_pad[t0+k, c] * band[k, m]

so PSUM partitions are channels (two 128-channel halves) and the free dim is
time.  Channel-major output means each partition's store is a long contiguous
run in a [C, L] DRAM tensor (host un-transposes at the end).

Precision/speed: the correctness gate is rel_err < 2e-2, so everything runs
in plain bf16 (measured ~3e-3 end-to-end): bf16 inputs, bf16 band, single
1-cyc/row matmul pass per (block, channel-half), fp32 PSUM accumulation, and
bf16 stores (host converts back to f32).  Relative to the fp32-accurate hi/lo
split this halves every DMA stream and cuts matmul passes 3x.

DMA batching: per-DMA overhead (~0.3-0.5us fixed + descriptor costs) made
many small loads the bottleneck, so blocks are materialized host-side with
their overlap ([NBLK, K, C], +8% bytes) and fetched J=7 blocks per DMA:
 - per supertile: ONE input load [128, J*C] bf16 (DRAM-side AP reordered
   "j p c -> p j c"; 512B contiguous runs)
 - per batch: ONE band load [128, NSUP*J*M] bf16 ("s p m -> p s m")
 - 14 matmuls per supertile (7 blocks x 2 channel halves) into psum
   [128, J*128] (block stride padded 118->128 so every matmul output is
   bank-aligned)
 - compact psum -> batch-wide SBUF tile [128, 4130] bf16 (f32->bf16 on
   VectorE for half 0, ScalarE for half 1)
 - per batch per half: ONE [128, 4096] bf16 store (8KB contiguous per
   partition, whole region contiguous in DRAM)
Engine/queue assignment is table-driven via CFG (tuned in CoreSim).
"""

import ml_dtypes
import numpy as np

import concourse.bass as bass  # noqa: F401  (engine handles hang off nc)
import concourse.mybir as mybir
import concourse.tile as tile
from concourse import bacc
from concourse.bass_utils import run_bass_kernel_spmd

B, L, C, T = 32, 4096, 256, 11
D = T // 2
N_CORES = 8
B_LOC = B // N_CORES            # 4 batches per core
M = 118                         # output rows per matmul block
K = M + T - 1                   # 128 = contraction rows per block
NBLK = -(-L // M)               # 35 blocks per batch
J = 7                           # blocks per supertile
NSUP = NBLK // J                # 5 supertiles per batch
SUP = M * J                     # 826 output rows per supertile
MP = 128                        # padded per-block psum stride (bank aligned)
LPAD = (NBLK - 1) * M + K       # 4140 padded input rows
LACC = NBLK * M                 # 4130 accumulated output rows per batch

_CACHE: dict = {}
LAST_RESULT = None  # BassKernelResults of the most recent run (for test.py)

# engine assignment knobs (sim-searchable): queues for input/band/store DMAs
# and engines for the two psum->SBUF compact copies
CFG = {
    "input_q": "sync",          # supertile input loads (SP HWDGE)
    "band_q": "gpsimd",         # per-batch band loads (SWDGE)
    "store_q0": "scalar",       # ch-half-0 store (ACT HWDGE)
    "store_q1": "gpsimd",       # ch-half-1 store (SWDGE)
    "copy0": "vector",          # ch-half-0 compact copy (DVE)
    # ch-half-1 compact copy: supertiles s < copy1_n_dve on DVE, rest ACT
    # (gpsimd can't touch PSUM, so copies can only go DVE/ACT)
    "copy1_n_dve": 2,
}


def _eng(nc, name):
    return {
        "sync": nc.sync,
        "scalar": nc.scalar,
        "vector": nc.vector,
        "gpsimd": nc.gpsimd,
        "tensor": nc.tensor,
    }[name]


def _build_nc(repeat: int = 1, bench: bool = False, cfg: dict | None = None):
    """Build the bass program. `repeat` re-runs the whole body N times and
    `bench=True` uses internal zero-filled DRAM inputs/outputs with only a
    tiny external "tick" output — both used only for benchmarking; the
    grading path uses repeat=1, bench=False."""
    cfg = {**CFG, **(cfg or {})}
    nc = bacc.Bacc(
        "TRN2",
        target_bir_lowering=False,
        debug=False,
        num_devices=N_CORES,
    )
    if bench:
        inp = nc.dram_tensor(
            "in_int", [B_LOC, NBLK, K, C], mybir.dt.bfloat16
        ).ap()
        band = nc.dram_tensor(
            "band_int", [B_LOC, NSUP, K, J * M], mybir.dt.bfloat16
        ).ap()
        outT = nc.dram_tensor("outT_int", [B_LOC, C, L], mybir.dt.bfloat16).ap()
        tick = nc.dram_tensor(
            "tick", [1, C], mybir.dt.bfloat16, kind="ExternalOutput"
        ).ap()
    else:
        inp = nc.dram_tensor(
            "in_blocks",
            [B_LOC, NBLK, K, C],
            mybir.dt.bfloat16,
            kind="ExternalInput",
        ).ap()
        band = nc.dram_tensor(
            "band",
            [B_LOC, NSUP, K, J * M],
            mybir.dt.bfloat16,
            kind="ExternalInput",
        ).ap()
        outT = nc.dram_tensor(
            "outT", [B_LOC, C, L], mybir.dt.bfloat16, kind="ExternalOutput"
        ).ap()
        tick = None

    with tile.TileContext(nc) as tc:
        with (
            tc.tile_pool(name="inp", bufs=4) as in_pool,
            tc.tile_pool(name="bnd", bufs=2) as bd_pool,
            tc.tile_pool(name="outp", bufs=2) as o_pool,
            tc.tile_pool(name="ps", bufs=4, space="PSUM") as ps_pool,
        ):
            if bench:
                # back every DRAM page with zeros once per run so reads are
                # real HBM traffic (unbacked-page reads measure absurdly
                # fast and would not represent the grading path)
                with tc.tile_pool(name="z", bufs=1) as z_pool:
                    z = z_pool.tile([K, SUP], mybir.dt.float32, tag="z")
                    nc.gpsimd.memset(z[:, :], 0.0)
                    for b in range(B_LOC):
                        for j in range(NBLK):
                            nc.sync.dma_start(
                                out=inp[b, j],
                                in_=z[:, : C // 2].bitcast(mybir.dt.bfloat16),
                            )
                        for s in range(NSUP):
                            nc.sync.dma_start(
                                out=band[b, s],
                                in_=z[:, : (J * M) // 2].bitcast(
                                    mybir.dt.bfloat16
                                ),
                            )
                        for ch in range(2):
                            for s in range(NSUP):
                                lo, hi = s * SUP, min((s + 1) * SUP, L)
                                nc.sync.dma_start(
                                    out=outT[b, ch * 128 : (ch + 1) * 128, lo:hi],
                                    in_=z[:, : (hi - lo) // 2].bitcast(
                                        mybir.dt.bfloat16
                                    ),
                                )

            q_in = _eng(nc, cfg["input_q"])
            q_bd = _eng(nc, cfg["band_q"])
            q_st = [_eng(nc, cfg["store_q0"]), _eng(nc, cfg["store_q1"])]
            e_c0 = _eng(nc, cfg["copy0"])
            n1 = cfg["copy1_n_dve"]

            def _copy(eng, dst, src):
                if eng is nc.scalar:
                    eng.copy(out=dst, in_=src)
                else:
                    eng.tensor_copy(out=dst, in_=src)

            for _rep in range(repeat):
                for b in range(B_LOC):
                    # batch-wide output accumulators (one per channel half)
                    o_ts = [
                        o_pool.tile(
                            [128, LACC],
                            mybir.dt.bfloat16,
                            tag=f"o{ch}",
                            name=f"o_t{ch}",
                        )
                        for ch in range(2)
                    ]
                    # ---- ONE band load per batch: [128, NSUP*J*M] ----
                    bd_t = bd_pool.tile(
                        [K, NSUP * J * M], mybir.dt.bfloat16, tag="bd"
                    )
                    q_bd.dma_start(
                        out=bd_t[:, :],
                        in_=band[b].rearrange("s p m -> p s m"),
                    )
                    for s in range(NSUP):
                        t0 = s * SUP
                        # ---- ONE input load per supertile: [128, J*C] ----
                        in_t = in_pool.tile(
                            [K, J * C], mybir.dt.bfloat16, tag="in"
                        )
                        q_in.dma_start(
                            out=in_t[:, :],
                            in_=inp[b, s * J : (s + 1) * J].rearrange(
                                "j p c -> p j c"
                            ),
                        )

                        # ---- matmuls: psum[c, m] per channel half ----
                        for ch in range(2):
                            ps = ps_pool.tile(
                                [128, J * MP], mybir.dt.float32, tag="ps"
                            )
                            for jj in range(J):
                                ih = in_t[
                                    :,
                                    jj * C + ch * 128 : jj * C + (ch + 1) * 128,
                                ]
                                bh = bd_t[
                                    :,
                                    s * SUP + jj * M : s * SUP + (jj + 1) * M,
                                ]
                                out_sl = ps[:, jj * MP : jj * MP + M]
                                nc.tensor.matmul(
                                    out_sl, ih, bh, start=True, stop=True
                                )
                            # ---- compact f32->bf16 copy into the batch
                            # accumulator ----
                            src = ps.rearrange("p (j m) -> p j m", j=J)[:, :, :M]
                            dst = o_ts[ch][:, t0 : t0 + SUP].rearrange(
                                "p (j m) -> p j m", j=J
                            )
                            if ch == 0:
                                e_c1 = e_c0
                            else:
                                e_c1 = nc.vector if s < n1 else nc.scalar
                            _copy(e_c0 if ch == 0 else e_c1, dst, src)

                    # ---- one big contiguous store per channel half ----
                    for ch in range(2):
                        q_st[ch].dma_start(
                            out=outT[b, ch * 128 : (ch + 1) * 128, :],
                            in_=o_ts[ch][:, :L],
                        )
                if tick is not None:
                    # flush both HWDGE queues: same-queue reads complete only
                    # after all prior writes on that queue
                    fl = o_pool.tile([2, C], mybir.dt.bfloat16, tag="fl")
                    nc.sync.dma_start(out=fl[0:1, :], in_=outT[0, 0:1, 0:C])
                    nc.scalar.dma_start(out=fl[1:2, :], in_=outT[0, 128:129, 0:C])
                    nc.sync.dma_start(out=tick[:, :], in_=fl[0:1, :])
                    nc.sync.dma_start(out=tick[:, 0:C], in_=fl[1:2, :])
    nc.compile()
    return nc


BF16 = ml_dtypes.bfloat16


_BLK_IDX = np.arange(NBLK)[:, None] * M + np.arange(K)[None, :]  # [NBLK, K]


def _prep_core(x: np.ndarray, w: np.ndarray):
    """x: [B_LOC, L, C] f32, w: [B_LOC, L, T] f32 -> (in_blocks, band), bf16.

    in_blocks materializes each matmul block's 128 contraction rows
    ([NBLK, K, C], overlapping windows, +8% bytes) so one DMA can fetch a
    whole supertile of blocks."""
    in_pad = np.zeros((B_LOC, LPAD, C), BF16)
    in_pad[:, D : D + L, :] = x.astype(BF16)
    in_blocks = np.ascontiguousarray(in_pad[:, _BLK_IDX, :])
    band16 = np.zeros((B_LOC, NBLK, K, M), BF16)
    jj, mm = np.meshgrid(np.arange(NBLK), np.arange(M), indexing="ij")
    tt = jj * M + mm
    v = tt < L
    jv, mv_, tv = jj[v], mm[v], tt[v]
    w16 = w.astype(BF16)
    for tau in range(T):
        band16[:, jv, mv_ + tau, mv_] = w16[:, tv, tau]
    # regroup into supertile layout [B_LOC, NSUP, K, J*M]
    band16 = np.ascontiguousarray(
        band16.reshape(B_LOC, NSUP, J, K, M).transpose(0, 1, 3, 2, 4)
    ).reshape(B_LOC, NSUP, K, J * M)
    return in_blocks, band16


def kernel(inputs: np.ndarray, weights: np.ndarray) -> np.ndarray:
    global LAST_RESULT
    inputs = np.ascontiguousarray(np.asarray(inputs, dtype=np.float32))
    weights = np.ascontiguousarray(np.asarray(weights, dtype=np.float32))
    assert inputs.shape == (B, L, C) and weights.shape == (B, L, T)

    if "nc" not in _CACHE:
        _CACHE["nc"] = _build_nc()
    nc = _CACHE["nc"]

    in_maps = []
    for c in range(N_CORES):
        sl = slice(c * B_LOC, (c + 1) * B_LOC)
        ip, bd = _prep_core(inputs[sl], weights[sl])
        in_maps.append({"in_blocks": ip, "band": bd})

    res = run_bass_kernel_spmd(nc, in_maps, core_ids=list(range(N_CORES)))
    LAST_RESULT = res
    # outputs come back channel-major bf16 [B_LOC, C, L]; un-transpose and
    # widen to f32 on host
    return np.ascontiguousarray(
        np.concatenate(
            [
                r["outT"].astype(np.float32).transpose(0, 2, 1)
                for r in res.results
            ],
            axis=0,
        )
    )


# revision 25
# speedup vs baseline: 1.2990x; 1.2990x over previous
"""Trainium2 Bass kernel for AttnApply (sliding-window weighted sum).

out[b, t, c] = sum_i padded[b, t+i, c] * weights[b, t, i]   (T=11, D=5 zero pad)

Strategy
--------
Pure data parallel over batch: 8 cores x 4 batches each.

Per core, the windowed sum is a banded matrix multiply on the TensorEngine.
For a time block of M=118 output rows starting at t0 (K = M+T-1 = 128):

    out[t0+m, c] = sum_k band[k, m] * in_pad[t0+k, c],   k in [0, 128)

with band[k, m] = w[t0+m, k-m] for 0 <= k-m < T (zero elsewhere); in_pad is
host zero-padded so edge blocks need no special casing.  Band matrices are
built host-side (cheap scatter of the small weights tensor).

The matmul runs with the INPUT tile as the stationary operand and the band as
the moving operand, producing the TRANSPOSED output in PSUM:

    psum[c, m] = sum_k in_pad[t0+k, c] * band[k, m]

so PSUM partitions are channels (two 128-channel halves) and the free dim is
time.  Channel-major output means each partition's store is a long contiguous
run in a [C, L] DRAM tensor (host un-transposes at the end).

Precision/speed: the correctness gate is rel_err < 2e-2, so everything runs
in plain bf16 (measured ~3e-3 end-to-end): bf16 inputs, bf16 band, single
1-cyc/row matmul pass per (block, channel-half), fp32 PSUM accumulation, and
bf16 stores (host converts back to f32).  Relative to the fp32-accurate hi/lo
split this halves every DMA stream and cuts matmul passes 3x.

DMA batching: per-DMA overhead (~0.3-0.5us fixed + descriptor costs) made
many small loads the bottleneck, so blocks are materialized host-side with
their overlap ([NBLK, K, C], +8% bytes) and fetched J=7 blocks per DMA:
 - per supertile: ONE input load [128, J*C] bf16 (partition-major DRAM
   layout, 3.5KB contiguous runs)
 - per batch: ONE band load [128, NSUP*J*M] bf16 (8KB contiguous runs)
 - 14 matmuls per supertile (7 blocks x 2 channel halves) into psum
   [128, J*128] (block stride padded 118->128 so every matmul output is
   bank-aligned)
 - compact psum -> batch-wide SBUF tile [128, 4130] bf16 (f32->bf16 on
   VectorE for half 0, ScalarE for half 1)
 - per batch per half: ONE [128, 4096] bf16 store (8KB contiguous per
   partition, whole region contiguous in DRAM)
Engine/queue assignment is table-driven via CFG (tuned in CoreSim).
"""

import ml_dtypes
import numpy as np

import concourse.bass as bass  # noqa: F401  (engine handles hang off nc)
import concourse.mybir as mybir
import concourse.tile as tile
from concourse import bacc
from concourse.bass_utils import run_bass_kernel_spmd

B, L, C, T = 32, 4096, 256, 11
D = T // 2
N_CORES = 8
B_LOC = B // N_CORES            # 4 batches per core
M = 118                         # output rows per matmul block
K = M + T - 1                   # 128 = contraction rows per block
NBLK = -(-L // M)               # 35 blocks per batch
J = 7                           # blocks per supertile
NSUP = NBLK // J                # 5 supertiles per batch
SUP = M * J                     # 826 output rows per supertile
MP = 128                        # padded per-block psum stride (bank aligned)
LPAD = (NBLK - 1) * M + K       # 4140 padded input rows
LACC = NBLK * M                 # 4130 accumulated output rows per batch

_CACHE: dict = {}
LAST_RESULT = None  # BassKernelResults of the most recent run (for test.py)

# engine assignment knobs (sim-searchable): queues for input/band/store DMAs
# and engines for the two psum->SBUF compact copies
CFG = {
    "input_q": "sync",          # supertile input loads (SP HWDGE)
    "band_q": "gpsimd",         # per-batch band loads (SWDGE)
    "store_q0": "scalar",       # ch-half-0 store (ACT HWDGE)
    "store_q1": "gpsimd",       # ch-half-1 store (SWDGE)
    "copy0": "vector",          # ch-half-0 compact copy (DVE)
    # ch-half-1 compact copy: supertiles s < copy1_n_dve on DVE, rest ACT
    # (gpsimd can't touch PSUM, so copies can only go DVE/ACT)
    "copy1_n_dve": 2,
    "in_bufs": 6,
    "o_bufs": 3,
    "bd_bufs": 3,
}


def _eng(nc, name):
    return {
        "sync": nc.sync,
        "scalar": nc.scalar,
        "vector": nc.vector,
        "gpsimd": nc.gpsimd,
        "tensor": nc.tensor,
    }[name]


def _build_nc(repeat: int = 1, bench: bool = False, cfg: dict | None = None):
    """Build the bass program. `repeat` re-runs the whole body N times and
    `bench=True` uses internal zero-filled DRAM inputs/outputs with only a
    tiny external "tick" output — both used only for benchmarking; the
    grading path uses repeat=1, bench=False."""
    cfg = {**CFG, **(cfg or {})}
    nc = bacc.Bacc(
        "TRN2",
        target_bir_lowering=False,
        debug=False,
        num_devices=N_CORES,
    )
    # partition-major DRAM layouts: contraction-row (=SBUF partition) dim
    # first, so each partition's DMA read is one long contiguous run
    # (3.5KB/8KB vs 512B — measured ~40% faster reads on this part)
    if bench:
        inp = nc.dram_tensor(
            "in_int", [B_LOC, K, NBLK, C], mybir.dt.bfloat16
        ).ap()
        band = nc.dram_tensor(
            "band_int", [B_LOC, K, NSUP * J * M], mybir.dt.bfloat16
        ).ap()
        outT = nc.dram_tensor("outT_int", [B_LOC, C, L], mybir.dt.bfloat16).ap()
        tick = nc.dram_tensor(
            "tick", [1, C], mybir.dt.bfloat16, kind="ExternalOutput"
        ).ap()
    else:
        inp = nc.dram_tensor(
            "in_blocks",
            [B_LOC, K, NBLK, C],
            mybir.dt.bfloat16,
            kind="ExternalInput",
        ).ap()
        band = nc.dram_tensor(
            "band",
            [B_LOC, K, NSUP * J * M],
            mybir.dt.bfloat16,
            kind="ExternalInput",
        ).ap()
        outT = nc.dram_tensor(
            "outT", [B_LOC, C, L], mybir.dt.bfloat16, kind="ExternalOutput"
        ).ap()
        tick = None

    with tile.TileContext(nc) as tc:
        with (
            tc.tile_pool(name="inp", bufs=cfg["in_bufs"]) as in_pool,
            tc.tile_pool(name="bnd", bufs=cfg["bd_bufs"]) as bd_pool,
            tc.tile_pool(name="outp", bufs=cfg["o_bufs"]) as o_pool,
            tc.tile_pool(name="ps", bufs=4, space="PSUM") as ps_pool,
        ):
            if bench:
                # back every DRAM page with zeros once per run so reads are
                # real HBM traffic (unbacked-page reads measure absurdly
                # fast and would not represent the grading path)
                with tc.tile_pool(name="z", bufs=1) as z_pool:
                    z = z_pool.tile([K, SUP], mybir.dt.float32, tag="z")
                    nc.gpsimd.memset(z[:, :], 0.0)
                    zw = SUP * 2  # bf16 elems per backing chunk
                    for b in range(B_LOC):
                        flat_in = inp[b].rearrange("p j c -> p (j c)")
                        for r0 in range(0, NBLK * C, zw):
                            cnt = min(zw, NBLK * C - r0)
                            nc.sync.dma_start(
                                out=flat_in[:, r0 : r0 + cnt],
                                in_=z[:, : cnt // 2].bitcast(mybir.dt.bfloat16),
                            )
                        for r0 in range(0, NSUP * J * M, zw):
                            cnt = min(zw, NSUP * J * M - r0)
                            nc.sync.dma_start(
                                out=band[b, :, r0 : r0 + cnt],
                                in_=z[:, : cnt // 2].bitcast(mybir.dt.bfloat16),
                            )
                        for ch in range(2):
                            for s in range(NSUP):
                                lo, hi = s * SUP, min((s + 1) * SUP, L)
                                nc.sync.dma_start(
                                    out=outT[b, ch * 128 : (ch + 1) * 128, lo:hi],
                                    in_=z[:, : (hi - lo) // 2].bitcast(
                                        mybir.dt.bfloat16
                                    ),
                                )

            q_in = _eng(nc, cfg["input_q"])
            q_bd = _eng(nc, cfg["band_q"])
            q_st = [_eng(nc, cfg["store_q0"]), _eng(nc, cfg["store_q1"])]
            e_c0 = _eng(nc, cfg["copy0"])
            n1 = cfg["copy1_n_dve"]

            def _copy(eng, dst, src):
                if eng is nc.scalar:
                    eng.copy(out=dst, in_=src)
                else:
                    eng.tensor_copy(out=dst, in_=src)

            for _rep in range(repeat):
                for b in range(B_LOC):
                    # batch-wide output accumulators (one per channel half)
                    o_ts = [
                        o_pool.tile(
                            [128, LACC],
                            mybir.dt.bfloat16,
                            tag=f"o{ch}",
                            name=f"o_t{ch}",
                        )
                        for ch in range(2)
                    ]
                    # ---- ONE band load per batch: [128, NSUP*J*M] ----
                    bd_t = bd_pool.tile(
                        [K, NSUP * J * M], mybir.dt.bfloat16, tag="bd"
                    )
                    q_bd.dma_start(out=bd_t[:, :], in_=band[b])
                    for s in range(NSUP):
                        t0 = s * SUP
                        # ---- ONE input load per supertile: [128, J*C] ----
                        in_t = in_pool.tile(
                            [K, J * C], mybir.dt.bfloat16, tag="in"
                        )
                        q_in.dma_start(
                            out=in_t[:, :],
                            in_=inp[b, :, s * J : (s + 1) * J, :],
                        )

                        # ---- matmuls: psum[c, m] per channel half ----
                        for ch in range(2):
                            ps = ps_pool.tile(
                                [128, J * MP], mybir.dt.float32, tag="ps"
                            )
                            for jj in range(J):
                                ih = in_t[
                                    :,
                                    jj * C + ch * 128 : jj * C + (ch + 1) * 128,
                                ]
                                bh = bd_t[
                                    :,
                                    s * SUP + jj * M : s * SUP + (jj + 1) * M,
                                ]
                                out_sl = ps[:, jj * MP : jj * MP + M]
                                nc.tensor.matmul(
                                    out_sl, ih, bh, start=True, stop=True
                                )
                            # ---- compact f32->bf16 copy into the batch
                            # accumulator ----
                            src = ps.rearrange("p (j m) -> p j m", j=J)[:, :, :M]
                            dst = o_ts[ch][:, t0 : t0 + SUP].rearrange(
                                "p (j m) -> p j m", j=J
                            )
                            if ch == 0:
                                e_c1 = e_c0
                            else:
                                e_c1 = nc.vector if s < n1 else nc.scalar
                            _copy(e_c0 if ch == 0 else e_c1, dst, src)

                    # ---- one big contiguous store per channel half ----
                    for ch in range(2):
                        q_st[ch].dma_start(
                            out=outT[b, ch * 128 : (ch + 1) * 128, :],
                            in_=o_ts[ch][:, :L],
                        )
                if tick is not None:
                    # flush both HWDGE queues: same-queue reads complete only
                    # after all prior writes on that queue
                    fl = o_pool.tile([2, C], mybir.dt.bfloat16, tag="fl")
                    nc.sync.dma_start(out=fl[0:1, :], in_=outT[0, 0:1, 0:C])
                    nc.scalar.dma_start(out=fl[1:2, :], in_=outT[0, 128:129, 0:C])
                    nc.sync.dma_start(out=tick[:, :], in_=fl[0:1, :])
                    nc.sync.dma_start(out=tick[:, 0:C], in_=fl[1:2, :])
    nc.compile()
    return nc


BF16 = ml_dtypes.bfloat16


_BLK_IDX = np.arange(NBLK)[:, None] * M + np.arange(K)[None, :]  # [NBLK, K]


def _prep_core(x: np.ndarray, w: np.ndarray):
    """x: [B_LOC, L, C] f32, w: [B_LOC, L, T] f32 -> (in_blocks, band), bf16.

    in_blocks materializes each matmul block's 128 contraction rows
    ([NBLK, K, C], overlapping windows, +8% bytes) so one DMA can fetch a
    whole supertile of blocks."""
    in_pad = np.zeros((B_LOC, LPAD, C), BF16)
    in_pad[:, D : D + L, :] = x.astype(BF16)
    # partition-major [B_LOC, K, NBLK, C]: row p of block j at [b, p, j, :]
    in_blocks = np.ascontiguousarray(
        in_pad[:, _BLK_IDX, :].transpose(0, 2, 1, 3)
    )
    band16 = np.zeros((B_LOC, NBLK, K, M), BF16)
    jj, mm = np.meshgrid(np.arange(NBLK), np.arange(M), indexing="ij")
    tt = jj * M + mm
    v = tt < L
    jv, mv_, tv = jj[v], mm[v], tt[v]
    w16 = w.astype(BF16)
    for tau in range(T):
        band16[:, jv, mv_ + tau, mv_] = w16[:, tv, tau]
    # partition-major supertile layout [B_LOC, K, NSUP*J*M]
    band16 = np.ascontiguousarray(
        band16.reshape(B_LOC, NSUP, J, K, M).transpose(0, 3, 1, 2, 4)
    ).reshape(B_LOC, K, NSUP * J * M)
    return in_blocks, band16


def kernel(inputs: np.ndarray, weights: np.ndarray) -> np.ndarray:
    global LAST_RESULT
    inputs = np.ascontiguousarray(np.asarray(inputs, dtype=np.float32))
    weights = np.ascontiguousarray(np.asarray(weights, dtype=np.float32))
    assert inputs.shape == (B, L, C) and weights.shape == (B, L, T)

    if "nc" not in _CACHE:
        _CACHE["nc"] = _build_nc()
    nc = _CACHE["nc"]

    in_maps = []
    for c in range(N_CORES):
        sl = slice(c * B_LOC, (c + 1) * B_LOC)
        ip, bd = _prep_core(inputs[sl], weights[sl])
        in_maps.append({"in_blocks": ip, "band": bd})

    res = run_bass_kernel_spmd(nc, in_maps, core_ids=list(range(N_CORES)))
    LAST_RESULT = res
    # outputs come back channel-major bf16 [B_LOC, C, L]; un-transpose and
    # widen to f32 on host
    return np.ascontiguousarray(
        np.concatenate(
            [
                r["outT"].astype(np.float32).transpose(0, 2, 1)
                for r in res.results
            ],
            axis=0,
        )
    )
